# revision 52
# baseline (speedup 1.0000x reference)
"""Trainium2 Bass kernel for nn_MultiHeadAttention (B=4, N=1024, D=1024, H=16).

Returns (out, attn) like the reference:
    qkv = x @ w_qkv + b_qkv; per-head attention with softmax; attn probs are an
    output; out = ctx @ w_proj + b_proj.

Sharding: tensor-parallel over heads — core c owns heads {2c, 2c+1}. Each core
computes its heads' q/k/v (from the full x), the full (B, 2, N, N) attention
block (written straight to HBM), and the head-sliced context. The context is
resharded head-split -> token-split with an 8-core AllToAll, after which each
core runs the projection for its 512 tokens against the full w_proj.

Layout choices:
  * The host passes x pre-transposed ([D, B*N]) so the PE (which contracts over
    the partition dim) can consume it directly; weights are pre-sliced/packed.
  * Scores are computed in both orientations: [n, m] for softmax + the attn
    output write, and [m, n] for the attn @ v matmul (rhs needs the contraction
    dim on partitions). A ones-row appended to v gives the softmax denominator
    for free in the same matmul.
  * All matmuls stream fp32 data as float32r (full-rate on TRN2 for moving
    dim >= 256; plain fp32 is 4 cycles/row). The BIR verifier requires fp32r
    matmul inputs to be *produced* as fp32r, so every producing instruction
    (DMA load or compute) writes through an fp32r-bitcast AP.
"""

import sys

sys.path.insert(0, "/opt/trn_rl_repo")

import numpy as np

B = 4
N = 1024
D = 1024
H = 16
HD = 64
NCORES = 8
HPC = H // NCORES          # heads per core = 2
E = 3 * HPC * HD           # per-core qkv feature count = 384
T = B * N                  # tokens = 4096
TB = T // NCORES           # tokens per core after reshard = 512
SCALE = 0.125              # 1/sqrt(HD)
_ONES = np.ones((128, 128), dtype=np.float32)

_CACHE = {}


def _build(collective=True):
    key = ("nc", collective)
    if key in _CACHE:
        return _CACHE[key]

    from contextlib import ExitStack

    import concourse.bacc as bacc
    import concourse.mybir as mybir
    import concourse.tile as tile
    from concourse.masks import make_identity

    f32 = mybir.dt.float32
    f32r = mybir.dt.float32r
    AF = mybir.ActivationFunctionType

    nc = bacc.Bacc("TRN2", target_bir_lowering=False, debug=False,
                   num_devices=NCORES)

    xT = nc.dram_tensor("xT", [D, T], f32, kind="ExternalInput")
    wqkv = nc.dram_tensor("wqkv", [D, E], f32, kind="ExternalInput")
    bqkv = nc.dram_tensor("bqkv", [E, 1], f32, kind="ExternalInput")
    wproj = nc.dram_tensor("wproj", [D, D], f32, kind="ExternalInput")
    bproj = nc.dram_tensor("bproj", [1, D], f32, kind="ExternalInput")
    ones_in = nc.dram_tensor("ones_in", [128, 128], f32, kind="ExternalInput")
    attn_o = nc.dram_tensor("attn_o", [B, HPC, N, N], f32, kind="ExternalOutput")
    out_o = nc.dram_tensor("out_o", [TB, D], f32, kind="ExternalOutput")

    with tile.TileContext(nc) as tc:
        glob = ExitStack()
        const_pool = glob.enter_context(tc.tile_pool(name="const", bufs=1))
        ident = const_pool.tile([128, 128], f32, name="ident")
        make_identity(nc, ident)
        ones_sb = const_pool.tile([128, 128], f32, name="ones_sb")
        nc.sync.dma_start(ones_sb[:].bitcast(f32r), ones_in[:].bitcast(f32r))
        ones_row = ones_sb[0:1, :]

        dram_pool = glob.enter_context(
            tc.tile_pool(name="dram", bufs=1, space="DRAM"))
        a2a_ins = [dram_pool.tile([NCORES, HD, TB], f32, name=f"a2a_in{h}")
                   for h in range(HPC)]
        a2a_outs = [dram_pool.tile([NCORES, HD, TB], f32, name=f"a2a_out{h}")
                    for h in range(HPC)]

        # q/k/v transposed, per e-tile: [128 feat, T]; feat = 2 heads x 64
        stack_qkv = ExitStack()
        qkvT_pool = stack_qkv.enter_context(tc.tile_pool(name="qkvT", bufs=1))
        qkvT = [[qkvT_pool.tile([128, N], f32, name=f"qkvT{e}b{b}")
                 for b in range(B)] for e in range(3)]

        # ---------------- phase 2: attention per (head, b) -------------------
        ph2 = ExitStack()
        vone_pool = ph2.enter_context(tc.tile_pool(name="vone", bufs=2))
        sc_ps_pool = ph2.enter_context(
            tc.tile_pool(name="scps", bufs=3, space="PSUM"))
        est_pool = ph2.enter_context(tc.tile_pool(name="est", bufs=4))
        pv_ps_pool = ph2.enter_context(
            tc.tile_pool(name="pvps", bufs=2, space="PSUM"))
        es_pool = ph2.enter_context(tc.tile_pool(name="es", bufs=4))
        den_pool = ph2.enter_context(tc.tile_pool(name="den", bufs=8))
        asb_pool = ph2.enter_context(tc.tile_pool(name="asb", bufs=2))
        rrow_pool = ph2.enter_context(tc.tile_pool(name="rrow", bufs=2))
        ctx_pool = ph2.enter_context(tc.tile_pool(name="ctxT", bufs=2))

        # ---------------- phase 1: qkvT = wqkv.T @ x.T (+ bias) --------------
        ph1 = ExitStack()
        xt_pool = ph1.enter_context(tc.tile_pool(name="xt", bufs=8))
        w_pool = ph1.enter_context(tc.tile_pool(name="wq", bufs=8))
        b_pool = ph1.enter_context(tc.tile_pool(name="bq", bufs=3))

        # interleave the first batch's x tiles with the weight tiles so the
        # k-th accumulation chain unblocks as soon as pair k lands
        ws, bqs, xts0 = [], [], []
        for k in range(8):
            t_x = xt_pool.tile([128, N], f32, name="xtile")
            nc.sync.dma_start(t_x[:].bitcast(f32r),
                              xT[128 * k:128 * (k + 1), 0:N].bitcast(f32r))
            xts0.append(t_x)
            t_w = w_pool.tile([128, E], f32, name="wtile")
            nc.sync.dma_start(t_w[:].bitcast(f32r),
                              wqkv[128 * k:128 * (k + 1), :].bitcast(f32r))
            ws.append(t_w)
        for e in range(3):
            t_b = b_pool.tile([128, 1], f32, name="bqtile")
            nc.sync.dma_start(t_b[:], bqkv[128 * e:128 * (e + 1), :])
            bqs.append(t_b)

        # x.T streamed per batch (quarter), so batch-0 attention can start
        # while later batches' qkv is still loading/computing
        def qkv_batch(b):
            if b == 0:
                xts = xts0
            else:
                xts = []
                for k in range(8):
                    t_x = xt_pool.tile([128, N], f32, name="xtile")
                    nc.sync.dma_start(
                        t_x[:].bitcast(f32r),
                        xT[128 * k:128 * (k + 1),
                           N * b:N * (b + 1)].bitcast(f32r))
                    xts.append(t_x)
            for tch in range(2):
                for e in range(3):
                    # borrow the (ramp-idle) PV pool's banks for the v chains
                    # so more qkv accumulations run concurrently
                    if e == 2:
                        ps = pv_ps_pool.tile([128, 512], f32, name="pvps")
                    else:
                        ps_full = sc_ps_pool.tile([128, 1024], f32, name="scps")
                        ps = ps_full[:, 0:512]
                    for k in range(8):
                        nc.tensor.matmul(
                            ps[:],
                            ws[k][:, 128 * e:128 * (e + 1)].bitcast(f32r),
                            xts[k][:, 512 * tch:512 * (tch + 1)].bitcast(f32r),
                            start=(k == 0), stop=(k == 7))
                    nc.vector.tensor_scalar_add(
                        qkvT[e][b][:, 512 * tch:512 * (tch + 1)].bitcast(f32r),
                        ps[:], bqs[e][:])

        def attn_bh(hp, b, s_first=False):
                p0 = HD * hp
                qT = qkvT[0][b][p0:p0 + HD, :]
                kT = qkvT[1][b][p0:p0 + HD, :]
                vT = qkvT[2][b][p0:p0 + HD, :]

                def do_pv_side():
                    # v tiles: 8x PE-transpose into one PSUM bank, single evict to
                    # [128, 8, 65] (64 v columns + a ones column per m-tile)
                    trp_full = sc_ps_pool.tile([128, 1024], f32, name="scps")
                    trp = trp_full[:, 0:512]
                    for mt in range(8):
                        nc.tensor.transpose(
                            trp[:, HD * mt:HD * (mt + 1)],
                            vT[:, 128 * mt:128 * (mt + 1)],
                            ident[p0:p0 + HD, p0:p0 + HD])
                    vone = vone_pool.tile([128, 8, HD + 1], f32, name="vone")
                    nc.vector.tensor_copy(
                        vone[:, :, 0:HD].bitcast(f32r),
                        trp[:].rearrange("p (a d) -> p a d", d=HD))
                    nc.vector.tensor_copy(vone[:, :, HD:HD + 1].bitcast(f32r),
                          ones_sb[:, 0:8].rearrange("p (a o) -> p a o", o=1))

                    # S^T (scores transposed) -> exp -> PV accumulation
                    pv0 = pv_ps_pool.tile([HD + 1, 512], f32, name="pvps")
                    pv1 = pv_ps_pool.tile([HD + 1, 512], f32, name="pvps")
                    for mt in range(8):
                        stp = sc_ps_pool.tile([128, 1024], f32, name="scps")
                        for half in range(2):
                            nc.tensor.matmul(
                                stp[:, 512 * half:512 * (half + 1)],
                                kT[:, 128 * mt:128 * (mt + 1)].bitcast(f32r),
                                qT[:, 512 * half:512 * (half + 1)].bitcast(f32r),
                                start=True, stop=True)
                        est = est_pool.tile([128, 1024], f32, name="est")
                        nc.scalar.activation(est[:].bitcast(f32r), stp[:],
                                             AF.Exp, scale=SCALE)
                        for half, pv in ((0, pv0), (1, pv1)):
                            nc.tensor.matmul(
                                pv[:],
                                vone[:, mt, :].bitcast(f32r),
                                est[:, 512 * half:512 * (half + 1)].bitcast(f32r),
                                start=(mt == 0), stop=(mt == 7))

                    # normalize context: rows 0..63 of pv are ctx^T, row 64 = denom
                    rrow = rrow_pool.tile([1, 1024], f32, name="rrow")
                    ctxT = ctx_pool.tile([HD, 1024], f32, name="ctxT")
                    for half, pv in ((0, pv0), (1, pv1)):
                        sl = slice(512 * half, 512 * (half + 1))
                        with nc.allow_low_precision(
                                reason="fp32r feed for factor broadcast"):
                            nc.vector.reciprocal(rrow[:, sl].bitcast(f32r),
                                                 pv[HD:HD + 1, :])
                        fps_full = sc_ps_pool.tile([128, 1024], f32, name="scps")
                        fps = fps_full[0:HD, 0:512]
                        nc.tensor.matmul(
                            fps,
                            ones_sb[0:1, 0:HD].bitcast(f32r),
                            rrow[0:1, sl].bitcast(f32r),
                            start=True, stop=True)
                        # tensor_tensor may read only ONE input from PSUM:
                        # stage the broadcast factor through SBUF first
                        fsb = rrow_pool.tile([HD, 512], f32, name="fsb")
                        nc.vector.tensor_copy(fsb[:], fps)
                        nc.vector.tensor_mul(ctxT[:, sl], pv[0:HD, :], fsb[:])
                    for half in range(2):
                        nc.sync.dma_start(
                            a2a_ins[hp][2 * b + half, :, :],
                            ctxT[:, 512 * half:512 * (half + 1)])

                def do_s_side():
                    # S (scores natural) -> exp+rowsum -> normalize -> attn out
                    G = 2  # n-tiles per output DMA
                    for grp in range(8 // G):
                        asb = asb_pool.tile([128, G * 1024], f32, name="asb")
                        for j in range(G):
                            nt = G * grp + j
                            sp = sc_ps_pool.tile([128, 1024], f32, name="scps")
                            for half in range(2):
                                nc.tensor.matmul(
                                    sp[:, 512 * half:512 * (half + 1)],
                                    qT[:, 128 * nt:128 * (nt + 1)].bitcast(f32r),
                                    kT[:, 512 * half:512 * (half + 1)].bitcast(f32r),
                                    start=True, stop=True)
                            esb = es_pool.tile([128, 1024], f32, name="esb")
                            den = den_pool.tile([128, 1], f32, name="den")
                            nc.scalar.activation(esb[:], sp[:], AF.Exp,
                                                 scale=SCALE, accum_out=den[:])
                            rcol = den_pool.tile([128, 1], f32, name="rcol")
                            nc.vector.reciprocal(rcol[:], den[:])
                            nc.vector.tensor_scalar_mul(
                                asb[:, 1024 * j:1024 * (j + 1)], esb[:], rcol[:])
                        nc.gpsimd.dma_start(
                            attn_o[b, hp, 128 * G * grp:128 * G * (grp + 1),
                                   :].rearrange("(a p) m -> p a m", p=128),
                            asb[:].rearrange("p (a m) -> p a m", a=G))

                if s_first:
                    do_s_side()
                    do_pv_side()
                else:
                    do_pv_side()
                    do_s_side()

        def reshard(hp):
            if collective:
                nc.gpsimd.collective_compute(
                    "AllToAll", mybir.AluOpType.bypass,
                    replica_groups=[list(range(NCORES))],
                    ins=[a2a_ins[hp].opt()], outs=[a2a_outs[hp].opt()])
            else:  # timing-sim variant: stand-in local copy, same bytes
                nc.sync.dma_start(a2a_outs[hp][:], a2a_ins[hp][:])

        # interleave: qkv(b) then attention(hp=0, b) so ACT/DVE start early
        for b in range(B):
            qkv_batch(b)
            attn_bh(0, b)
        ph1.close()

        # phase-4 pools opened early so wproj / ctx-even / bias stream in
        # during the hp=1 attention sweep
        ph4 = ExitStack()
        wp_pool = ph4.enter_context(tc.tile_pool(name="wp", bufs=16))
        cf_pool = ph4.enter_context(tc.tile_pool(name="cf", bufs=8))
        bias_pool = ph4.enter_context(tc.tile_pool(name="bias", bufs=1))
        osb_pool = ph4.enter_context(tc.tile_pool(name="osb", bufs=2))

        reshard(0)

        # preloads overlapping the hp=1 sweep
        cfs, wps = [], {}
        for k in range(8):
            cf = cf_pool.tile([128, TB], f32, name="cftile")
            nc.sync.dma_start(cf[0:HD, :].bitcast(f32r),
                              a2a_outs[0][k, :, :].bitcast(f32r))
            cfs.append(cf)
        for half in range(2):
            for k in range(8):
                wp = wp_pool.tile([128, 512], f32, name="wptile")
                nc.sync.dma_start(
                    wp[:].bitcast(f32r),
                    wproj[128 * k:128 * (k + 1),
                          512 * half:512 * (half + 1)].bitcast(f32r))
                wps[(half, k)] = wp
        bprow = bias_pool.tile([1, D], f32, name="bprow")
        nc.sync.dma_start(bprow[:].bitcast(f32r), bproj[:].bitcast(f32r))

        for b in range(B):
            attn_bh(1, b)
        reshard(1)

        # ---------------- phase 4: out = ctx @ wproj + bproj -----------------
        for k in range(8):
            nc.sync.dma_start(cfs[k][HD:128, :].bitcast(f32r),
                              a2a_outs[1][k, :, :].bitcast(f32r))
        bias_sb = bias_pool.tile([128, D], f32, name="bias_sb")
        for half in range(2):
            bps_full = sc_ps_pool.tile([128, 1024], f32, name="scps")
            bps = bps_full[:, 0:512]
            nc.tensor.matmul(
                bps,
                ones_sb[0:1, 0:128].bitcast(f32r),
                bprow[0:1, 512 * half:512 * (half + 1)].bitcast(f32r),
                start=True, stop=True)
            nc.vector.tensor_copy(bias_sb[:, 512 * half:512 * (half + 1)],
                                  bps)

        for half in range(2):
            for tt in range(TB // 128):
                pp_full = sc_ps_pool.tile([128, 1024], f32, name="scps")
                pp = pp_full[:, 0:512]
                for k in range(8):
                    nc.tensor.matmul(
                        pp,
                        cfs[k][:, 128 * tt:128 * (tt + 1)].bitcast(f32r),
                        wps[(half, k)][:].bitcast(f32r),
                        start=(k == 0), stop=(k == 7))
                osb = osb_pool.tile([128, 512], f32, name="osb")
                nc.vector.tensor_add(
                    osb[:], pp,
                    bias_sb[:, 512 * half:512 * (half + 1)])
                nc.sync.dma_start(
                    out_o[128 * tt:128 * (tt + 1),
                          512 * half:512 * (half + 1)], osb[:])

        ph4.close()
        ph2.close()
        stack_qkv.close()
        glob.close()

    nc.compile()
    _CACHE[key] = nc
    return nc


def _make_in_maps(x, w_qkv, b_qkv, w_proj, b_proj):
    x = np.ascontiguousarray(np.asarray(x, dtype=np.float32))
    w_qkv = np.asarray(w_qkv, dtype=np.float32)
    b_qkv = np.asarray(b_qkv, dtype=np.float32)
    w_proj = np.ascontiguousarray(np.asarray(w_proj, dtype=np.float32))
    b_proj = np.asarray(b_proj, dtype=np.float32).reshape(1, D).copy()

    xT = np.ascontiguousarray(x.reshape(T, D).T)
    in_maps = []
    for c in range(NCORES):
        heads = [HPC * c + i for i in range(HPC)]
        cols = [w_qkv[:, s * D + h * HD:s * D + (h + 1) * HD]
                for s in range(3) for h in heads]
        wqkv_c = np.ascontiguousarray(np.concatenate(cols, axis=1))
        bq_c = np.concatenate(
            [b_qkv[s * D + h * HD:s * D + (h + 1) * HD]
             for s in range(3) for h in heads]).reshape(E, 1)
        in_maps.append({
            "xT": xT,
            "wqkv": wqkv_c,
            "bqkv": np.ascontiguousarray(bq_c),
            "wproj": w_proj,
            "bproj": b_proj,
            "ones_in": _ONES,
        })
    return in_maps


def run(inputs, **hw_kwargs):
    """Build + run on 8 cores; returns (out, attn, BassKernelResults)."""
    from concourse import bass_utils

    nc = _build()
    in_maps = _make_in_maps(inputs["x"], inputs["w_qkv"], inputs["b_qkv"],
                            inputs["w_proj"], inputs["b_proj"])
    res = bass_utils.run_bass_kernel_spmd(
        nc, in_maps, core_ids=list(range(NCORES)), **hw_kwargs)
    attn = np.concatenate(
        [res.results[c]["attn_o"] for c in range(NCORES)], axis=1)
    out = np.concatenate(
        [res.results[c]["out_o"] for c in range(NCORES)],
        axis=0).reshape(B, N, D)
    return out, attn, res


def kernel(**inputs):
    out, attn, _ = run(inputs)
    return out, attn

